# revision 1
# baseline (speedup 1.0000x reference)
"""Trainium2 Bass kernel for nn_DecoderLSTM_noAttention (greedy decode LSTM).

Strategy (8 NeuronCores, SPMD, zero collectives):
- Data-parallel over batch: core c owns batch rows [8c, 8c+8) end-to-end.
  The sequential 31-step greedy decode runs fully per-core; no cross-core
  communication at all (collectives on this axon per-core topology are
  host-emulated and cost ~270ms each).
- LSTM gates on-chip in fp32: x rows gathered from the embedding table by
  indirect DMA, then x@W_ih.T + h@W_hh.T + (b_ih+b_hh) accumulated in one
  PSUM group.
- FC (h @ W_fc.T + b_fc) is memory-bound: W_fc.T is streamed from DRAM in
  fp16 every step (32.8 MB/step in 16 double-buffered 2MB slabs).  Logits
  are written out in fp16 (abs err ~3e-4 << tolerance).
- Greedy argmax must match the fp32 reference ordering exactly, so the
  fp16 scan only nominates candidates: per-4000-column segment top-8
  (DVE max/max_index), then one max + match_replace pass extracts the
  scan top-4 (duplicate-value safe, order irrelevant).  The 4 candidate
  rows of the original fp32 W_fc (+bias) are indirect-gathered and their
  exact fp32 dots with the full-precision h decide the token.
- Output: each core writes fp16 logits [31, 8, 32000]; the host assembles
  the full fp32 [64, 32, 32000] (t=0 stays zero).
"""
import numpy as np

import concourse.bass as bass
import concourse.bacc as bacc
import concourse.tile as tile
from concourse import mybir
from concourse.bass_utils import run_bass_kernel_spmd
from concourse.masks import make_identity

F32 = mybir.dt.float32
F16 = mybir.dt.float16
I32 = mybir.dt.int32
U16 = mybir.dt.uint16
AF = mybir.ActivationFunctionType
OP = mybir.AluOpType

B = 64          # global batch
H = 512         # hidden = embed
V = 32000       # vocab
T = 32          # max_len
NPIX = 196
NCORES = 8
BL = B // NCORES      # local batch rows per core
NSTEPS = T - 1
GD = 2048             # gate dim

NSLAB = 16            # W_fc.T stream slabs per step
SLABW = V // NSLAB    # 2000 columns per slab
NSEG = 8              # scan-max segments
SEGW = V // NSEG      # 4000 columns per segment
NCAND = 4             # exact-recheck candidates
WROWW = 520           # padded fp32 W_fc row (512 weights + bias + pad)

# chunk offsets within one slab (PSUM bank is 512 fp32 wide)
SLAB_CHUNKS = [(0, 512), (512, 512), (1024, 512), (1536, 464)]

_CACHE = {}


def _build_nc(nsteps=NSTEPS, out_slots=NSTEPS, loop_reps=1):
    """loop_reps>1 wraps the decode body in a HW loop (timing-only variant:
    LSTM state goes stale across loop iterations, but per-step cost is
    identical, which amplifies the measurable signal loop_reps-fold)."""
    nc = bacc.Bacc("TRN2", target_bir_lowering=False, debug=False,
                   num_devices=NCORES)

    # ---- DRAM parameters ----
    emb_d = nc.dram_tensor("emb", [V, H], F32, kind="ExternalInput")
    wrow_d = nc.dram_tensor("wrow", [V, WROWW], F32, kind="ExternalInput")
    whi_d = nc.dram_tensor("whiT", [NSLAB, 128, 4, SLABW], F16,
                           kind="ExternalInput")
    wih_d = nc.dram_tensor("wihT", [4, 128, GD], F32, kind="ExternalInput")
    whh_d = nc.dram_tensor("whhT", [4, 128, GD], F32, kind="ExternalInput")
    winit_d = nc.dram_tensor("winitT", [4, 128, 1024], F32, kind="ExternalInput")
    bgate_d = nc.dram_tensor("bgate", [1, GD], F32, kind="ExternalInput")
    binit_d = nc.dram_tensor("binit", [1, 1024], F32, kind="ExternalInput")
    enc_d = nc.dram_tensor("enc", [13, 128, H], F32, kind="ExternalInput")
    blk_d = nc.dram_tensor("blkdiag", [128, 13 * 8], F32, kind="ExternalInput")
    tok0_d = nc.dram_tensor("tok0", [BL, 1], I32, kind="ExternalInput")

    out_d = nc.dram_tensor("logits", [out_slots, BL, V], F16,
                           kind="ExternalOutput")

    with tile.TileContext(nc) as tc:
        import contextlib
        with contextlib.ExitStack() as ctx:
            # bufs=1 for loop variants: keeps tag->buffer assignment identical
            # across HW-loop iterations (odd per-iteration allocation counts
            # with bufs=2 flip buffer parity across the back-edge -> deadlock)
            sb = 1 if loop_reps > 1 else 2
            const = ctx.enter_context(tc.tile_pool(name="const", bufs=1))
            work = ctx.enter_context(tc.tile_pool(name="work", bufs=1))
            hc = ctx.enter_context(tc.tile_pool(name="hc", bufs=sb))
            small = ctx.enter_context(tc.tile_pool(name="small", bufs=sb))
            lgp = ctx.enter_context(tc.tile_pool(name="lgp", bufs=2))
            stream = ctx.enter_context(tc.tile_pool(name="stream", bufs=4))
            ptr = ctx.enter_context(tc.tile_pool(name="ptr", bufs=1, space="PSUM"))
            pg = ctx.enter_context(tc.tile_pool(name="pg", bufs=1, space="PSUM"))
            pfc = ctx.enter_context(tc.tile_pool(name="pfc", bufs=3, space="PSUM"))

            # ---- constants / weights into SBUF ----
            ident = const.tile([BL, BL], F32)
            make_identity(nc, ident[:])
            ones1 = const.tile([1, BL], F32)
            nc.vector.memset(ones1[:], 1.0)
            segbase = const.tile([BL, NSEG * 8], F32)
            for j in range(NSEG):
                nc.vector.memset(segbase[:, j * 8:(j + 1) * 8], float(j * SEGW))

            wih = []
            whh = []
            for k in range(4):
                w = const.tile([128, GD], F32, tag=f"wih{k}")
                nc.sync.dma_start(w[:], wih_d[k])
                wih.append(w)
                w = const.tile([128, GD], F32, tag=f"whh{k}")
                nc.sync.dma_start(w[:], whh_d[k])
                whh.append(w)
            bgate = const.tile([1, GD], F32)
            nc.sync.dma_start(bgate[:], bgate_d[:])
            binit = const.tile([1, 1024], F32)
            nc.sync.dma_start(binit[:], binit_d[:])
            blk = work.tile([128, 13 * 8], F32, tag="blk")
            nc.sync.dma_start(blk[:], blk_d[:])

            def transpose_to(src, dst_tile):
                """src: SBUF [BL, 512] fp32 -> dst SBUF [128, 4*BL]."""
                for k in range(4):
                    pt = ptr.tile([128, BL], F32, tag="ptr")
                    nc.tensor.transpose(
                        out=pt[:], in_=src[:, k * 128:(k + 1) * 128],
                        identity=ident[:])
                    nc.scalar.copy(dst_tile[:, k * BL:(k + 1) * BL], pt[:])

            # ================= phase 0: h0/c0 from encoder mean =================
            psum0 = pfc.tile([BL, H], F32, tag="pfc")
            for k in range(13):
                et = work.tile([128, H], F32, tag="enc")
                nc.sync.dma_start(et[:], enc_d[k])
                nc.tensor.matmul(
                    psum0[:], lhsT=blk[:, k * 8:(k + 1) * 8], rhs=et[:],
                    start=(k == 0), stop=(k == 12))
            sums = work.tile([BL, H], F32, tag="sums")
            nc.scalar.copy(sums[:], psum0[:])

            sumT = work.tile([128, 4 * BL], F32, tag="hT")
            transpose_to(sums, sumT)

            for n in range(2):
                ph = pfc.tile([BL, 512], F32, tag="pfc")
                for k in range(4):
                    wi = work.tile([128, 1024], F32, tag="winit")
                    nc.sync.dma_start(wi[:], winit_d[k])
                    nc.tensor.matmul(
                        ph[:], lhsT=sumT[:, k * BL:(k + 1) * BL],
                        rhs=wi[:, n * 512:(n + 1) * 512],
                        start=(k == 0), stop=False)
                nc.tensor.matmul(
                    ph[:], lhsT=ones1[:],
                    rhs=binit[:, n * 512:(n + 1) * 512],
                    start=False, stop=True)
                dst = hc.tile([BL, H], F32, tag=("h" if n == 0 else "c"))
                nc.scalar.copy(dst[:], ph[:])
                if n == 0:
                    h_cur = dst
                else:
                    c_cur = dst
            hT = work.tile([128, 4 * BL], F32, tag="hT")
            transpose_to(h_cur, hT)

            tok = small.tile([BL, 1], I32, tag="tok")
            nc.sync.dma_start(tok[:], tok0_d[:])

            # ================= decode steps =================
            loop_cm = (tc.For_i(0, loop_reps) if loop_reps > 1
                       else contextlib.nullcontext())
            with loop_cm:
                for t in range(nsteps):
                    # --- x = embedding[tok]  (indirect row gather) ---
                    x = work.tile([BL, H], F32, tag="x")
                    nc.gpsimd.indirect_dma_start(
                        out=x[:], out_offset=None, in_=emb_d[:],
                        in_offset=bass.IndirectOffsetOnAxis(ap=tok[:, :1], axis=0))
                    xT = work.tile([128, 4 * BL], F32, tag="xT")
                    transpose_to(x, xT)

                    # --- gates = x@W_ih.T + h@W_hh.T + (b_ih+b_hh) in one PSUM ---
                    pgt = pg.tile([BL, GD], F32, tag="pg")
                    for n in range(4):
                        ns = slice(n * 512, (n + 1) * 512)
                        for k in range(4):
                            nc.tensor.matmul(
                                pgt[:, ns], lhsT=hT[:, k * BL:(k + 1) * BL],
                                rhs=whh[k][:, ns], start=(k == 0), stop=False)
                        for k in range(4):
                            nc.tensor.matmul(
                                pgt[:, ns], lhsT=xT[:, k * BL:(k + 1) * BL],
                                rhs=wih[k][:, ns], start=False, stop=False)
                        nc.tensor.matmul(
                            pgt[:, ns], lhsT=ones1[:], rhs=bgate[:, ns],
                            start=False, stop=True)

                    # --- pointwise LSTM ---
                    sig_if = work.tile([BL, 1024], F32, tag="sigif")
                    nc.scalar.activation(sig_if[:], pgt[:, 0:1024], AF.Sigmoid)
                    tng = work.tile([BL, 512], F32, tag="tng")
                    nc.scalar.activation(tng[:], pgt[:, 1024:1536], AF.Tanh)
                    sgo = work.tile([BL, 512], F32, tag="sgo")
                    nc.scalar.activation(sgo[:], pgt[:, 1536:2048], AF.Sigmoid)

                    t1 = work.tile([BL, 512], F32, tag="t1")
                    nc.vector.tensor_mul(t1[:], sig_if[:, 0:512], tng[:])
                    t2 = work.tile([BL, 512], F32, tag="t2")
                    nc.vector.tensor_mul(t2[:], sig_if[:, 512:1024], c_cur[:])
                    c_new = hc.tile([BL, H], F32, tag="c")
                    nc.vector.tensor_add(c_new[:], t2[:], t1[:])
                    tc2 = work.tile([BL, 512], F32, tag="tc2")
                    nc.scalar.activation(tc2[:], c_new[:], AF.Tanh)
                    h_new = hc.tile([BL, H], F32, tag="h")
                    nc.vector.tensor_mul(h_new[:], sgo[:], tc2[:])
                    c_cur = c_new

                    hT = work.tile([128, 4 * BL], F32, tag="hT")
                    transpose_to(h_new, hT)
                    hhiT = work.tile([128, 4 * BL], F16, tag="hhiT")
                    nc.vector.tensor_copy(hhiT[:], hT[:])

                    # --- FC scan: stream W_fc.T fp16, per-segment logits tile,
                    #     top-8 per segment, segment DMA'd straight out ---
                    cands = small.tile([BL, NSEG * 8], F16, tag="cands")
                    cidx = small.tile([BL, NSEG * 8], U16, tag="cidx")
                    ot = out_d[min(t, out_slots - 1)]
                    spseg = NSLAB // NSEG          # slabs per segment
                    for s in range(NSLAB):
                        st = stream.tile([128, 4, SLABW], F16, tag="st")
                        nc.sync.dma_start(st[:], whi_d[s])
                        if s % spseg == 0:
                            Lseg = lgp.tile([BL, SEGW], F16, tag="L")
                        lbase = (s % spseg) * SLABW
                        for off, w in SLAB_CHUNKS:
                            pf = pfc.tile([BL, 512], F32, tag="pfc")
                            for k in range(4):
                                nc.tensor.matmul(
                                    pf[:, :w], lhsT=hhiT[:, k * BL:(k + 1) * BL],
                                    rhs=st[:, k, off:off + w],
                                    start=(k == 0), stop=(k == 3))
                            nc.scalar.copy(Lseg[:, lbase + off:lbase + off + w],
                                           pf[:, :w])
                        if (s + 1) % spseg == 0:
                            j = s // spseg
                            js = slice(j * 8, (j + 1) * 8)
                            nc.vector.max(out=cands[:, js], in_=Lseg[:])
                            nc.vector.max_index(
                                out=cidx[:, js], in_max=cands[:, js],
                                in_values=Lseg[:])
                            nc.sync.dma_start(
                                ot[:, j * SEGW:(j + 1) * SEGW], Lseg[:])

                    # --- scan top-4 candidates (knockout on [BL, 64]) ---
                    cv = small.tile([BL, NSEG * 8], F32, tag="cv")
                    nc.vector.tensor_copy(cv[:], cands[:])
                    gidxf = small.tile([BL, NSEG * 8], F32, tag="gidxf")
                    nc.vector.tensor_copy(gidxf[:], cidx[:])
                    nc.vector.tensor_add(gidxf[:], gidxf[:], segbase[:])
                    gneg = small.tile([BL, NSEG * 8], F32, tag="gneg")
                    nc.vector.tensor_scalar(
                        out=gneg[:], in0=gidxf[:], scalar1=-1.0, scalar2=48000.0,
                        op0=OP.mult, op1=OP.add)
                    # top-4 candidate extraction without knockout rounds:
                    # max() yields the top-8 values descending; match_replace
                    # marks one occurrence per top-4 value (duplicate-safe),
                    # and a second max over masked gneg recovers the 4 ids
                    # (order irrelevant -- the exact recheck re-ranks them).
                    m8 = small.tile([BL, 8], F32, tag="m8")
                    nc.vector.max(out=m8[:], in_=cv[:])
                    m4pad = small.tile([BL, 8], F32, tag="m4pad")
                    nc.vector.memset(m4pad[:], 1e30)
                    nc.vector.tensor_copy(m4pad[:, 0:NCAND], m8[:, 0:NCAND])
                    cv2 = small.tile([BL, NSEG * 8], F32, tag="cv2")
                    nc.vector.match_replace(
                        out=cv2[:], in_to_replace=m4pad[:], in_values=cv[:],
                        imm_value=-1e30)
                    msk = small.tile([BL, NSEG * 8], F32, tag="msk")
                    nc.vector.tensor_scalar(
                        out=msk[:], in0=cv2[:], scalar1=-1e30, scalar2=None,
                        op0=OP.is_equal)
                    sel = small.tile([BL, NSEG * 8], F32, tag="sel")
                    nc.vector.tensor_mul(sel[:], msk[:], gneg[:])
                    w8 = small.tile([BL, 8], F32, tag="w8")
                    nc.vector.max(out=w8[:], in_=sel[:])
                    gid4 = small.tile([BL, NCAND], F32, tag="gid4")
                    nc.vector.tensor_scalar(
                        out=gid4[:], in0=w8[:, 0:NCAND], scalar1=-1.0,
                        scalar2=48000.0, op0=OP.mult, op1=OP.add)
                    cid = small.tile([BL, NCAND], I32, tag="cid")
                    nc.vector.tensor_copy(cid[:], gid4[:])
                    wcand = work.tile([BL, NCAND, WROWW], F32, tag="wcand")
                    for j in range(NCAND):
                        nc.gpsimd.indirect_dma_start(
                            out=wcand[:, j], out_offset=None, in_=wrow_d[:],
                            in_offset=bass.IndirectOffsetOnAxis(
                                ap=cid[:, j:j + 1], axis=0))
                    ex = small.tile([BL, 8], F32, tag="ex")
                    nc.vector.memset(ex[:], -1e30)
                    prod = work.tile([BL, 512], F32, tag="prod")
                    for j in range(NCAND):
                        nc.vector.tensor_mul(prod[:], wcand[:, j, 0:512], h_new[:])
                        nc.vector.tensor_reduce(
                            out=ex[:, j:j + 1], in_=prod[:],
                            axis=mybir.AxisListType.X, op=OP.add)
                    for j in range(NCAND):
                        nc.vector.tensor_add(ex[:, j:j + 1], ex[:, j:j + 1],
                                             wcand[:, j, 512:513])

                    # --- winner (value desc, then smallest id) -> next token ---
                    em8 = small.tile([BL, 8], F32, tag="em8")
                    nc.vector.max(out=em8[:], in_=ex[:])
                    emsk = small.tile([BL, NCAND], F32, tag="emsk")
                    nc.vector.tensor_scalar(
                        out=emsk[:], in0=ex[:, 0:NCAND], scalar1=em8[:, 0:1],
                        scalar2=None, op0=OP.is_equal)
                    egneg = small.tile([BL, NCAND], F32, tag="egneg")
                    nc.vector.tensor_scalar(
                        out=egneg[:], in0=gid4[:], scalar1=-1.0, scalar2=48000.0,
                        op0=OP.mult, op1=OP.add)
                    esel = small.tile([BL, 8], F32, tag="esel")
                    nc.vector.memset(esel[:], 0.0)
                    nc.vector.tensor_mul(esel[:, 0:NCAND], emsk[:], egneg[:])
                    ew8 = small.tile([BL, 8], F32, tag="ew8")
                    nc.vector.max(out=ew8[:], in_=esel[:])
                    tokf = small.tile([BL, 1], F32, tag="tokf")
                    nc.vector.tensor_scalar(
                        out=tokf[:], in0=ew8[:, 0:1], scalar1=-1.0, scalar2=48000.0,
                        op0=OP.mult, op1=OP.add)
                    tok = small.tile([BL, 1], I32, tag="tok")
                    nc.vector.tensor_copy(tok[:], tokf[:])

    nc.compile()
    return nc


def _prep_inputs(inputs):
    enc = np.ascontiguousarray(np.asarray(inputs["encoder_outputs"], np.float32))
    captions = np.asarray(inputs["captions"])
    emb = np.ascontiguousarray(np.asarray(inputs["embedding"], np.float32))
    W_ih = np.asarray(inputs["W_ih"], np.float32)
    b_ih = np.asarray(inputs["b_ih"], np.float32)
    W_hh = np.asarray(inputs["W_hh"], np.float32)
    b_hh = np.asarray(inputs["b_hh"], np.float32)
    W_fc = np.asarray(inputs["W_fc"], np.float32)
    b_fc = np.asarray(inputs["b_fc"], np.float32)
    W_init_h = np.asarray(inputs["W_init_h"], np.float32)
    b_init_h = np.asarray(inputs["b_init_h"], np.float32)
    W_init_c = np.asarray(inputs["W_init_c"], np.float32)
    b_init_c = np.asarray(inputs["b_init_c"], np.float32)

    wrow = np.zeros((V, WROWW), np.float32)
    wrow[:, 0:512] = W_fc
    wrow[:, 512] = b_fc
    wT16 = W_fc.T.astype(np.float16)                       # [512, V]
    whi = np.ascontiguousarray(
        wT16.reshape(4, 128, NSLAB, SLABW).transpose(2, 1, 0, 3))
    wihT = np.ascontiguousarray(W_ih.T.reshape(4, 128, GD))
    whhT = np.ascontiguousarray(W_hh.T.reshape(4, 128, GD))
    winitT = np.ascontiguousarray(
        (np.concatenate([W_init_h, W_init_c], axis=0) / np.float32(NPIX))
        .T.reshape(4, 128, 1024))
    bgate = (b_ih + b_hh).reshape(1, GD).astype(np.float32)
    binit = np.concatenate([b_init_h, b_init_c]).reshape(1, 1024)

    blkd = np.zeros((128, 13 * 8), np.float32)
    for k in range(13):
        for i in range(128):
            r = k * 128 + i
            if r < BL * NPIX:
                blkd[i, k * 8 + r // NPIX] = 1.0

    in_maps = []
    for c in range(NCORES):
        enc_c = enc[c * BL:(c + 1) * BL].reshape(BL * NPIX, H)
        enc_pad = np.zeros((13 * 128, H), np.float32)
        enc_pad[:BL * NPIX] = enc_c
        tok0 = np.ascontiguousarray(
            captions[c * BL:(c + 1) * BL, 0].astype(np.int32).reshape(BL, 1))
        in_maps.append({
            "emb": emb,
            "wrow": wrow,
            "whiT": whi,
            "wihT": wihT,
            "whhT": whhT,
            "winitT": winitT,
            "bgate": bgate,
            "binit": binit,
            "enc": enc_pad.reshape(13, 128, H),
            "blkdiag": blkd,
            "tok0": tok0,
        })
    return in_maps


def kernel(**inputs) -> np.ndarray:
    if "nc" not in _CACHE:
        _CACHE["nc"] = _build_nc()
    nc = _CACHE["nc"]
    in_maps = _prep_inputs(inputs)
    res = run_bass_kernel_spmd(nc, in_maps, list(range(NCORES)))
    out = np.zeros((B, T, V), np.float32)
    for c in range(NCORES):
        lg = res.results[c]["logits"][:NSTEPS]     # [31, BL, V] fp16
        out[c * BL:(c + 1) * BL, 1:, :] = lg.transpose(1, 0, 2)
    b_fc = np.asarray(inputs["b_fc"], np.float32)
    if b_fc.any():
        # scan logits are computed without the FC bias (it is ordering-exact
        # in the on-chip recheck); fold it into the full output here
        out[:, 1:, :] += b_fc
    return out



# revision 10
# speedup vs baseline: 1.5092x; 1.5092x over previous
"""Trainium2 Bass kernel for nn_DecoderLSTM_noAttention (greedy decode LSTM).

Strategy (8 NeuronCores, SPMD, zero collectives):
- Data-parallel over batch: core c owns batch rows [8c, 8c+8) end-to-end.
  The sequential 31-step greedy decode runs fully per-core.
- W_fc.T is RESIDENT in SBUF as fp8 e3m4 (x32 scaled): 16 MB, no per-step
  HBM streaming at all.  Scan logits (x32) are written out in fp16; the
  host divides by 32.  e3m4 quantization keeps max output error ~1.2% of
  the reference absmax (< 2e-2 tolerance).
- Greedy argmax must match the fp32 reference ordering exactly, so the
  fp8 scan only nominates candidates: per-4000-column segment top-8
  (DVE max/max_index), then one max + match_replace pass extracts the
  scan top-4; the 4 candidate rows of the original fp32 W_fc (+bias) are
  indirect-gathered and their exact fp32 dots with the full-precision h
  decide the token.
- x-side gate matmul is folded into a precomputed DRAM table
  xw = embedding @ W_ih.T + b_ih + b_hh  [V, 2048]; per step one indirect
  row gather replaces gather+transpose+matmuls.
- h-side gates h @ W_hh.T run as float32r (1 cyc/row vs 4 for fp32).
- Output: each core writes fp16 logits [31, 8, 32000] (x32); the host
  assembles the full fp32 [64, 32, 32000] (t=0 stays zero).
"""
import numpy as np

import concourse.bass as bass
import concourse.bacc as bacc
import concourse.tile as tile
from concourse import mybir
from concourse.bass_utils import run_bass_kernel_spmd
from concourse.masks import make_identity

F32 = mybir.dt.float32
F32R = mybir.dt.float32r
F16 = mybir.dt.float16
F8E3 = mybir.dt.float8e3
I32 = mybir.dt.int32
U16 = mybir.dt.uint16
AF = mybir.ActivationFunctionType
OP = mybir.AluOpType

B = 64          # global batch
H = 512         # hidden = embed
V = 32000       # vocab
T = 32          # max_len
NPIX = 196
NCORES = 8
BL = B // NCORES      # local batch rows per core
NSTEPS = T - 1
GD = 2048             # gate dim

WSCALE = 32.0         # fp8 table scale; logits come out x32
CHUNK = 512           # psum chunk (= one fp32 bank)
GROUPW = 2048         # psum group = 4 banks
NGROUP = (V + GROUPW - 1) // GROUPW   # 16 (last group 1280 wide)
NSEG = NGROUP         # scan-max segment == psum group
SEGW = GROUPW         # segment width (last segment 1280)
NCAND = 4             # exact-recheck candidates
WROWW = 520           # padded fp32 W_fc row (512 weights + bias + pad)

_CACHE = {}


def _build_nc(nsteps=NSTEPS, out_slots=NSTEPS, loop_reps=1):
    """loop_reps>1 wraps the decode body in a HW loop (timing-only variant:
    LSTM state goes stale across loop iterations, but per-step cost is
    identical, which amplifies the measurable signal loop_reps-fold)."""
    nc = bacc.Bacc("TRN2", target_bir_lowering=False, debug=False,
                   num_devices=NCORES)

    # ---- DRAM parameters ----
    wfc8_d = nc.dram_tensor("wfc8", [4, 128, V], F8E3, kind="ExternalInput")
    xw_d = nc.dram_tensor("xw", [V, GD], F32R, kind="ExternalInput")
    wrow_d = nc.dram_tensor("wrow", [V, WROWW], F32, kind="ExternalInput")
    whh_d = nc.dram_tensor("whhT", [4, 128, GD], F32R, kind="ExternalInput")
    winit_d = nc.dram_tensor("winitT", [4, 128, 1024], F32, kind="ExternalInput")
    binit_d = nc.dram_tensor("binit", [1, 1024], F32, kind="ExternalInput")
    enc_d = nc.dram_tensor("enc", [13, 128, H], F32, kind="ExternalInput")
    blk_d = nc.dram_tensor("blkdiag", [128, 13 * 8], F32, kind="ExternalInput")
    tok0_d = nc.dram_tensor("tok0", [BL, 1], I32, kind="ExternalInput")

    out_d = nc.dram_tensor("logits", [out_slots, BL, V], F16,
                           kind="ExternalOutput")

    with tile.TileContext(nc) as tc:
        import contextlib
        with contextlib.ExitStack() as ctx:
            const = ctx.enter_context(tc.tile_pool(name="const", bufs=1))
            work = ctx.enter_context(tc.tile_pool(name="work", bufs=1))
            hc = ctx.enter_context(tc.tile_pool(name="hc", bufs=1))
            small = ctx.enter_context(tc.tile_pool(name="small", bufs=1))
            lgp = ctx.enter_context(tc.tile_pool(name="lgp", bufs=2))
            ps = ctx.enter_context(tc.tile_pool(name="ps", bufs=2, space="PSUM"))

            # ---- constants / weights into SBUF ----
            ident = const.tile([BL, BL], F32)
            make_identity(nc, ident[:])
            ones1 = const.tile([1, BL], F32)
            nc.vector.memset(ones1[:], 1.0)
            segbase = const.tile([BL, NSEG * 8], F32)
            for j in range(NSEG):
                nc.vector.memset(segbase[:, j * 8:(j + 1) * 8], float(j * GROUPW))

            wfc8 = []
            for k in range(4):
                w8 = const.tile([128, V], F8E3, tag=f"wfc8{k}")
                nc.sync.dma_start(w8[:], wfc8_d[k])
                wfc8.append(w8)
            whh = []
            for k in range(4):
                w = const.tile([128, GD], F32R, tag=f"whh{k}")
                nc.sync.dma_start(w[:], whh_d[k])
                whh.append(w)
            ident_r = const.tile([BL, BL], F32R)
            nc.vector.tensor_copy(ident_r[:], ident[:])

            def transpose_to(src, dst_tile, pt):
                """src: SBUF [BL, 512] fp32 -> dst SBUF [128, 4*BL].
                pt: PSUM tile [128, >=4*BL]."""
                for k in range(4):
                    nc.tensor.transpose(
                        out=pt[:, k * BL:(k + 1) * BL],
                        in_=src[:, k * 128:(k + 1) * 128],
                        identity=ident[:])
                nc.scalar.copy(dst_tile[:], pt[:, 0:4 * BL])

            # ================= phase 0: h0/c0 from encoder mean =============
            ph0cm = tc.tile_pool(name="ph0", bufs=1)
            ph0 = ph0cm.__enter__()
            binit = ph0.tile([1, 1024], F32, tag="binit")
            nc.sync.dma_start(binit[:], binit_d[:])
            blk = ph0.tile([128, 13 * 8], F32, tag="blk")
            nc.sync.dma_start(blk[:], blk_d[:])
            pst = ps.tile([128, GD], F32, tag="ps")
            for k in range(13):
                et = ph0.tile([128, H], F32, tag="enc")
                nc.sync.dma_start(et[:], enc_d[k])
                nc.tensor.matmul(
                    pst[0:BL, 0:H], lhsT=blk[:, k * 8:(k + 1) * 8], rhs=et[:],
                    start=(k == 0), stop=(k == 12))
            sums = ph0.tile([BL, H], F32, tag="sums")
            nc.scalar.copy(sums[:], pst[0:BL, 0:H])

            pst = ps.tile([128, GD], F32, tag="ps")
            sumT = ph0.tile([128, 4 * BL], F32, tag="sumT")
            transpose_to(sums, sumT, pst)

            pst = ps.tile([128, GD], F32, tag="ps")
            for n in range(2):
                sl = slice(n * 512, (n + 1) * 512)
                for k in range(4):
                    wi = ph0.tile([128, 512], F32, tag="winit")
                    nc.sync.dma_start(wi[:], winit_d[k][:, n * 512:(n + 1) * 512])
                    nc.tensor.matmul(
                        pst[0:BL, sl], lhsT=sumT[:, k * BL:(k + 1) * BL],
                        rhs=wi[:],
                        start=(k == 0), stop=False)
                nc.tensor.matmul(
                    pst[0:BL, sl], lhsT=ones1[:],
                    rhs=binit[:, n * 512:(n + 1) * 512],
                    start=False, stop=True)
            h_cur = hc.tile([BL, H], F32, tag="h")
            nc.scalar.copy(h_cur[:], pst[0:BL, 0:512])
            c_cur = hc.tile([BL, H], F32, tag="c")
            nc.scalar.copy(c_cur[:], pst[0:BL, 512:1024])

            pst = ps.tile([128, GD], F32, tag="ps")
            hT = work.tile([128, 4 * BL], F32R, tag="hT")
            transpose_to(h_cur, hT, pst)
            hT16 = work.tile([128, 4 * BL], F16, tag="hT16")
            nc.vector.tensor_copy(hT16[:], hT[:])

            tok = small.tile([BL, 1], I32, tag="tok")
            nc.sync.dma_start(tok[:], tok0_d[:])
            ph0cm.__exit__(None, None, None)

            # ================= decode steps =================
            loop_cm = (tc.For_i(0, loop_reps) if loop_reps > 1
                       else contextlib.nullcontext())
            with loop_cm:
                for t in range(nsteps):
                    # --- xw = (emb@W_ih.T + b_ih + b_hh)[tok]  (row gather) ---
                    xw = work.tile([BL, GD], F32R, tag="xw")
                    nc.gpsimd.indirect_dma_start(
                        out=xw[:], out_offset=None, in_=xw_d[:],
                        in_offset=bass.IndirectOffsetOnAxis(ap=tok[:, :1], axis=0))
                    # actw: activation outputs early-step, wcand gather
                    # late-step (disjoint lifetimes share one buffer)
                    actw = work.tile([BL, 4 * WROWW], F32, tag="actw")

                    # --- gates = xw + h@W_hh.T in one PSUM (float32r) ---
                    pgt = ps.tile([128, GD], F32, tag="ps")
                    for n in range(4):
                        ns = slice(n * 512, (n + 1) * 512)
                        for k in range(4):
                            nc.tensor.matmul(
                                pgt[0:BL, ns],
                                lhsT=hT[:, k * BL:(k + 1) * BL],
                                rhs=whh[k][:, ns],
                                start=(k == 0), stop=False)
                        nc.tensor.matmul(
                            pgt[0:BL, ns], lhsT=ident_r[:],
                            rhs=xw[:, ns],
                            start=False, stop=True)

                    # --- pointwise LSTM (gate order i,f,g,o) ---
                    sig_if = actw[:, 0:1024]
                    nc.scalar.activation(sig_if, pgt[0:BL, 0:1024], AF.Sigmoid)
                    tng = actw[:, 1024:1536]
                    nc.scalar.activation(tng, pgt[0:BL, 1024:1536], AF.Tanh)
                    sgo = actw[:, 1536:2048]
                    nc.scalar.activation(sgo, pgt[0:BL, 1536:2048], AF.Sigmoid)

                    t1 = work.tile([BL, 512], F32, tag="t1")
                    nc.vector.tensor_mul(t1[:], sig_if[:, 0:512], tng)
                    t2 = work.tile([BL, 512], F32, tag="t2")
                    nc.vector.tensor_mul(t2[:], sig_if[:, 512:1024], c_cur[:])
                    c_new = hc.tile([BL, H], F32, tag="c")
                    nc.vector.tensor_add(c_new[:], t2[:], t1[:])
                    tc2 = work.tile([BL, 512], F32, tag="t1")
                    nc.scalar.activation(tc2[:], c_new[:], AF.Tanh)
                    h_new = hc.tile([BL, H], F32, tag="h")
                    nc.vector.tensor_mul(h_new[:], sgo, tc2[:])
                    c_cur = c_new

                    pst = ps.tile([128, GD], F32, tag="ps")
                    hT = work.tile([128, 4 * BL], F32R, tag="hT")
                    transpose_to(h_new, hT, pst)
                    hT16 = work.tile([128, 4 * BL], F16, tag="hT16")
                    nc.vector.tensor_copy(hT16[:], hT[:])

                    # --- FC scan vs resident fp8 table; per-segment top-8 ---
                    cands = small.tile([BL, NSEG * 8], F16, tag="cands")
                    cidx = small.tile([BL, NSEG * 8], U16, tag="cidx")
                    ot = out_d[min(t, out_slots - 1)]
                    for g in range(NGROUP):
                        wlo = g * GROUPW
                        wwid = min(GROUPW, V - wlo)
                        pf = ps.tile([128, GD], F32, tag="ps")
                        co = 0
                        while co < wwid:
                            cw = min(CHUNK, wwid - co)
                            cs = slice(co, co + cw)
                            for k in range(4):
                                nc.tensor.matmul(
                                    pf[0:BL, cs],
                                    lhsT=hT16[:, k * BL:(k + 1) * BL],
                                    rhs=wfc8[k][:, wlo + co:wlo + co + cw],
                                    start=(k == 0), stop=(k == 3))
                            co += cw
                        Lseg = lgp.tile([BL, SEGW], F16, tag="L")
                        nc.scalar.copy(Lseg[:, 0:wwid], pf[0:BL, 0:wwid])
                        js = slice(g * 8, (g + 1) * 8)
                        nc.vector.max(out=cands[:, js], in_=Lseg[:, 0:wwid])
                        nc.vector.max_index(
                            out=cidx[:, js], in_max=cands[:, js],
                            in_values=Lseg[:, 0:wwid])
                        nc.sync.dma_start(
                            ot[:, wlo:wlo + wwid], Lseg[:, 0:wwid])

                    # --- scan top-4 candidates (knockout on [BL, 64]) ---
                    cv = small.tile([BL, NSEG * 8], F32, tag="cv")
                    nc.vector.tensor_copy(cv[:], cands[:])
                    gidxf = small.tile([BL, NSEG * 8], F32, tag="gidxf")
                    nc.vector.tensor_copy(gidxf[:], cidx[:])
                    nc.vector.tensor_add(gidxf[:], gidxf[:], segbase[:])
                    gneg = small.tile([BL, NSEG * 8], F32, tag="gneg")
                    nc.vector.tensor_scalar(
                        out=gneg[:], in0=gidxf[:], scalar1=-1.0, scalar2=48000.0,
                        op0=OP.mult, op1=OP.add)
                    # max() yields top-8 values descending; match_replace
                    # marks one occurrence per top-4 value (duplicate-safe),
                    # and a second max over masked gneg recovers the 4 ids.
                    m8 = small.tile([BL, 8], F32, tag="m8")
                    nc.vector.max(out=m8[:], in_=cv[:])
                    m4pad = small.tile([BL, 8], F32, tag="m4pad")
                    nc.vector.memset(m4pad[:], 1e30)
                    nc.vector.tensor_copy(m4pad[:, 0:NCAND], m8[:, 0:NCAND])
                    cv2 = small.tile([BL, NSEG * 8], F32, tag="cv2")
                    nc.vector.match_replace(
                        out=cv2[:], in_to_replace=m4pad[:], in_values=cv[:],
                        imm_value=-1e30)
                    msk = small.tile([BL, NSEG * 8], F32, tag="msk")
                    nc.vector.tensor_scalar(
                        out=msk[:], in0=cv2[:], scalar1=-1e30, scalar2=None,
                        op0=OP.is_equal)
                    sel = small.tile([BL, NSEG * 8], F32, tag="sel")
                    nc.vector.tensor_mul(sel[:], msk[:], gneg[:])
                    w8 = small.tile([BL, 8], F32, tag="w8")
                    nc.vector.max(out=w8[:], in_=sel[:])
                    gid4 = small.tile([BL, NCAND], F32, tag="gid4")
                    nc.vector.tensor_scalar(
                        out=gid4[:], in0=w8[:, 0:NCAND], scalar1=-1.0,
                        scalar2=48000.0, op0=OP.mult, op1=OP.add)
                    cid = small.tile([BL, NCAND], I32, tag="cid")
                    nc.vector.tensor_copy(cid[:], gid4[:])
                    wcand = actw
                    for j in range(NCAND):
                        nc.gpsimd.indirect_dma_start(
                            out=wcand[:, j * WROWW:(j + 1) * WROWW],
                            out_offset=None, in_=wrow_d[:],
                            in_offset=bass.IndirectOffsetOnAxis(
                                ap=cid[:, j:j + 1], axis=0))
                    # exact fp32 dots (+bias as reduce init) on DVE
                    ex = small.tile([BL, 8], F32, tag="ex")
                    nc.vector.memset(ex[:], -1e30)
                    prod = work.tile([BL, 512], F32, tag="prod")
                    for j in range(NCAND):
                        nc.vector.tensor_mul(
                            prod[:], wcand[:, j * WROWW:j * WROWW + 512],
                            h_new[:])
                        nc.vector.tensor_reduce(
                            out=ex[:, j:j + 1], in_=prod[:],
                            axis=mybir.AxisListType.X, op=OP.add)
                    for j in range(NCAND):
                        nc.vector.tensor_add(
                            ex[:, j:j + 1], ex[:, j:j + 1],
                            wcand[:, j * WROWW + 512:j * WROWW + 513])

                    # --- winner (value desc, then smallest id) -> next token ---
                    em8 = small.tile([BL, 8], F32, tag="em8")
                    nc.vector.max(out=em8[:], in_=ex[:])
                    emsk = small.tile([BL, NCAND], F32, tag="emsk")
                    nc.vector.tensor_scalar(
                        out=emsk[:], in0=ex[:, 0:NCAND], scalar1=em8[:, 0:1],
                        scalar2=None, op0=OP.is_equal)
                    egneg = small.tile([BL, NCAND], F32, tag="egneg")
                    nc.vector.tensor_scalar(
                        out=egneg[:], in0=gid4[:], scalar1=-1.0, scalar2=48000.0,
                        op0=OP.mult, op1=OP.add)
                    esel = small.tile([BL, 8], F32, tag="esel")
                    nc.vector.memset(esel[:], 0.0)
                    nc.vector.tensor_mul(esel[:, 0:NCAND], emsk[:], egneg[:])
                    ew8 = small.tile([BL, 8], F32, tag="ew8")
                    nc.vector.max(out=ew8[:], in_=esel[:])
                    tokf = small.tile([BL, 1], F32, tag="tokf")
                    nc.vector.tensor_scalar(
                        out=tokf[:], in0=ew8[:, 0:1], scalar1=-1.0, scalar2=48000.0,
                        op0=OP.mult, op1=OP.add)
                    tok = small.tile([BL, 1], I32, tag="tok")
                    nc.vector.tensor_copy(tok[:], tokf[:])

    nc.compile()
    return nc


def _prep_inputs(inputs):
    enc = np.ascontiguousarray(np.asarray(inputs["encoder_outputs"], np.float32))
    captions = np.asarray(inputs["captions"])
    emb = np.asarray(inputs["embedding"], np.float32)
    W_ih = np.asarray(inputs["W_ih"], np.float32)
    b_ih = np.asarray(inputs["b_ih"], np.float32)
    W_hh = np.asarray(inputs["W_hh"], np.float32)
    b_hh = np.asarray(inputs["b_hh"], np.float32)
    W_fc = np.asarray(inputs["W_fc"], np.float32)
    b_fc = np.asarray(inputs["b_fc"], np.float32)
    W_init_h = np.asarray(inputs["W_init_h"], np.float32)
    b_init_h = np.asarray(inputs["b_init_h"], np.float32)
    W_init_c = np.asarray(inputs["W_init_c"], np.float32)
    b_init_c = np.asarray(inputs["b_init_c"], np.float32)

    import ml_dtypes
    wfc8 = np.ascontiguousarray(
        np.clip(W_fc.T * np.float32(WSCALE), -15.5, 15.5)
        .reshape(4, 128, V)).astype(ml_dtypes.float8_e3m4)

    def trunc_r(a):
        return (a.view(np.uint32) & np.uint32(0xFFFFFC00)).view(np.float32)

    xw = np.ascontiguousarray(
        trunc_r((emb @ W_ih.T + b_ih + b_hh).astype(np.float32)))

    wrow = np.zeros((V, WROWW), np.float32)
    wrow[:, 0:512] = W_fc
    wrow[:, 512] = b_fc
    whhT = np.ascontiguousarray(trunc_r(W_hh.T).reshape(4, 128, GD))
    winitT = np.ascontiguousarray(
        (np.concatenate([W_init_h, W_init_c], axis=0) / np.float32(NPIX))
        .T.reshape(4, 128, 1024))
    binit = np.concatenate([b_init_h, b_init_c]).reshape(1, 1024)

    blkd = np.zeros((128, 13 * 8), np.float32)
    for k in range(13):
        for i in range(128):
            r = k * 128 + i
            if r < BL * NPIX:
                blkd[i, k * 8 + r // NPIX] = 1.0

    in_maps = []
    for c in range(NCORES):
        enc_c = enc[c * BL:(c + 1) * BL].reshape(BL * NPIX, H)
        enc_pad = np.zeros((13 * 128, H), np.float32)
        enc_pad[:BL * NPIX] = enc_c
        tok0 = np.ascontiguousarray(
            captions[c * BL:(c + 1) * BL, 0].astype(np.int32).reshape(BL, 1))
        in_maps.append({
            "wfc8": wfc8,
            "xw": xw,
            "wrow": wrow,
            "whhT": whhT,
            "winitT": winitT,
            "binit": binit,
            "enc": enc_pad.reshape(13, 128, H),
            "blkdiag": blkd,
            "tok0": tok0,
        })
    return in_maps


def kernel(**inputs) -> np.ndarray:
    if "nc" not in _CACHE:
        _CACHE["nc"] = _build_nc()
    nc = _CACHE["nc"]
    in_maps = _prep_inputs(inputs)
    res = run_bass_kernel_spmd(nc, in_maps, list(range(NCORES)))
    out = np.zeros((B, T, V), np.float32)
    inv = np.float32(1.0 / WSCALE)
    for c in range(NCORES):
        lg = res.results[c]["logits"][:NSTEPS]     # [31, BL, V] fp16 (x32)
        out[c * BL:(c + 1) * BL, 1:, :] = lg.transpose(1, 0, 2)
    out[:, 1:, :] *= inv
    b_fc = np.asarray(inputs["b_fc"], np.float32)
    if b_fc.any():
        # scan logits are computed without the FC bias (it is ordering-exact
        # in the on-chip recheck); fold it into the full output here
        out[:, 1:, :] += b_fc
    return out


# revision 13
# speedup vs baseline: 1.5398x; 1.0203x over previous
"""Trainium2 Bass kernel for nn_DecoderLSTM_noAttention (greedy decode LSTM).

Strategy (8 NeuronCores, SPMD, zero collectives):
- Data-parallel over batch: core c owns batch rows [8c, 8c+8) end-to-end.
  The sequential 31-step greedy decode runs fully per-core.
- W_fc.T is RESIDENT in SBUF as fp8 e3m4 (x32 scaled): 16 MB, no per-step
  HBM streaming at all.  Scan logits (x32) are written out in fp16; the
  host divides by 32.  e3m4 quantization keeps max output error ~1.2% of
  the reference absmax (< 2e-2 tolerance).
- Greedy argmax must match the fp32 reference ordering exactly, so the
  fp8 scan only nominates candidates: per-4000-column segment top-8
  (DVE max/max_index), then one max + match_replace pass extracts the
  scan top-4; the 4 candidate rows of the original fp32 W_fc (+bias) are
  indirect-gathered and their exact fp32 dots with the full-precision h
  decide the token.
- x-side gate matmul is folded into a precomputed DRAM table
  xw = embedding @ W_ih.T + b_ih + b_hh  [V, 2048]; per step one indirect
  row gather replaces gather+transpose+matmuls.
- h-side gates h @ W_hh.T run as float32r (1 cyc/row vs 4 for fp32).
- Output: each core writes fp16 logits [31, 8, 32000] (x32); the host
  assembles the full fp32 [64, 32, 32000] (t=0 stays zero).
"""
import numpy as np

import concourse.bass as bass
import concourse.bacc as bacc
import concourse.tile as tile
from concourse import mybir
from concourse.bass_utils import run_bass_kernel_spmd
from concourse.masks import make_identity

F32 = mybir.dt.float32
F32R = mybir.dt.float32r
F16 = mybir.dt.float16
F8E3 = mybir.dt.float8e3
I32 = mybir.dt.int32
U16 = mybir.dt.uint16
AF = mybir.ActivationFunctionType
OP = mybir.AluOpType

B = 64          # global batch
H = 512         # hidden = embed
V = 32000       # vocab
T = 32          # max_len
NPIX = 196
NCORES = 8
BL = B // NCORES      # local batch rows per core
NSTEPS = T - 1
GD = 2048             # gate dim

WSCALE = 32.0         # fp8 table scale; logits come out x32
CHUNK = 512           # psum chunk (= one fp32 bank)
GROUPW = 2048         # psum group = 4 banks
NGROUP = (V + GROUPW - 1) // GROUPW   # 16 (last group 1280 wide)
NSEG = NGROUP         # scan-max segment == psum group
SEGW = GROUPW         # segment width (last segment 1280)
NCAND = 4             # exact-recheck candidates
WROWW = 520           # padded fp32 W_fc row (512 weights + bias + pad)

_CACHE = {}


def _build_nc(nsteps=NSTEPS, out_slots=NSTEPS, loop_reps=1):
    """loop_reps>1 wraps the decode body in a HW loop (timing-only variant:
    LSTM state goes stale across loop iterations, but per-step cost is
    identical, which amplifies the measurable signal loop_reps-fold)."""
    nc = bacc.Bacc("TRN2", target_bir_lowering=False, debug=False,
                   num_devices=NCORES)

    # ---- DRAM parameters ----
    wfc8_d = nc.dram_tensor("wfc8", [4, 128, V], F8E3, kind="ExternalInput")
    xw_d = nc.dram_tensor("xw", [V, GD], F32R, kind="ExternalInput")
    wrow_d = nc.dram_tensor("wrow", [V, WROWW], F32, kind="ExternalInput")
    whh_d = nc.dram_tensor("whhT", [4, 128, GD], F32R, kind="ExternalInput")
    winit_d = nc.dram_tensor("winitT", [4, 128, 1024], F32, kind="ExternalInput")
    binit_d = nc.dram_tensor("binit", [1, 1024], F32, kind="ExternalInput")
    enc_d = nc.dram_tensor("enc", [13, 128, H], F32, kind="ExternalInput")
    blk_d = nc.dram_tensor("blkdiag", [128, 13 * 8], F32, kind="ExternalInput")
    tok0_d = nc.dram_tensor("tok0", [BL, 1], I32, kind="ExternalInput")

    out_d = nc.dram_tensor("logits", [out_slots, BL, V], F16,
                           kind="ExternalOutput")
    scr_d = nc.dram_tensor("scr", [BL * 1000, 32], F16, kind="Internal")

    with tile.TileContext(nc) as tc:
        import contextlib
        with contextlib.ExitStack() as ctx:
            const = ctx.enter_context(tc.tile_pool(name="const", bufs=1))
            work = ctx.enter_context(tc.tile_pool(name="work", bufs=1))
            hc = ctx.enter_context(tc.tile_pool(name="hc", bufs=1))
            small = ctx.enter_context(tc.tile_pool(name="small", bufs=1))
            ps = ctx.enter_context(tc.tile_pool(name="ps", bufs=2, space="PSUM"))

            # ---- constants / weights into SBUF ----
            ident = const.tile([BL, BL], F32)
            make_identity(nc, ident[:])
            ones1 = const.tile([1, BL], F32)
            nc.vector.memset(ones1[:], 1.0)
            rowbase_i = const.tile([BL, 1], I32)
            nc.gpsimd.iota(rowbase_i[:], pattern=[[0, 1]], base=0,
                           channel_multiplier=1000)
            rowbase = const.tile([BL, 1], F32)
            nc.vector.tensor_copy(rowbase[:], rowbase_i[:])
            pay_i = const.tile([BL, 1000], mybir.dt.int16)
            nc.gpsimd.iota(pay_i[:], pattern=[[-1, 1000]], base=2048,
                           channel_multiplier=0)
            payload = const.tile([BL, 1000], F16)
            nc.vector.tensor_copy(payload[:], pay_i[:])
            Mt = work.tile([BL, 1000, 1], F16, tag="M")
            nc.vector.memset(Mt[:], -49152.0)
            fence = small.tile([BL, NSEG], F32, tag="fence")
            scrv = scr_d[:].rearrange("(b r) c -> b (r c)", b=BL)

            wfc8 = []
            for k in range(4):
                w8 = const.tile([128, V], F8E3, tag=f"wfc8{k}")
                nc.sync.dma_start(w8[:], wfc8_d[k])
                wfc8.append(w8)
            whh = []
            for k in range(4):
                w = const.tile([128, GD], F32R, tag=f"whh{k}")
                nc.sync.dma_start(w[:], whh_d[k])
                whh.append(w)
            ident_r = const.tile([BL, BL], F32R)
            nc.vector.tensor_copy(ident_r[:], ident[:])

            def transpose_to(src, dst_tile, pt):
                """src: SBUF [BL, 512] fp32 -> dst SBUF [128, 4*BL].
                pt: PSUM tile [128, >=4*BL]."""
                for k in range(4):
                    nc.tensor.transpose(
                        out=pt[:, k * BL:(k + 1) * BL],
                        in_=src[:, k * 128:(k + 1) * 128],
                        identity=ident[:])
                nc.scalar.copy(dst_tile[:], pt[:, 0:4 * BL])

            # ================= phase 0: h0/c0 from encoder mean =============
            ph0cm = tc.tile_pool(name="ph0", bufs=1)
            ph0 = ph0cm.__enter__()
            binit = ph0.tile([1, 1024], F32, tag="binit")
            nc.sync.dma_start(binit[:], binit_d[:])
            blk = ph0.tile([128, 13 * 8], F32, tag="blk")
            nc.sync.dma_start(blk[:], blk_d[:])
            pst = ps.tile([128, GD], F32, tag="ps")
            for k in range(13):
                et = ph0.tile([128, H], F32, tag="enc")
                nc.sync.dma_start(et[:], enc_d[k])
                nc.tensor.matmul(
                    pst[0:BL, 0:H], lhsT=blk[:, k * 8:(k + 1) * 8], rhs=et[:],
                    start=(k == 0), stop=(k == 12))
            sums = ph0.tile([BL, H], F32, tag="sums")
            nc.scalar.copy(sums[:], pst[0:BL, 0:H])

            pst = ps.tile([128, GD], F32, tag="ps")
            sumT = ph0.tile([128, 4 * BL], F32, tag="sumT")
            transpose_to(sums, sumT, pst)

            pst = ps.tile([128, GD], F32, tag="ps")
            for n in range(2):
                sl = slice(n * 512, (n + 1) * 512)
                for k in range(4):
                    wi = ph0.tile([128, 512], F32, tag="winit")
                    nc.sync.dma_start(wi[:], winit_d[k][:, n * 512:(n + 1) * 512])
                    nc.tensor.matmul(
                        pst[0:BL, sl], lhsT=sumT[:, k * BL:(k + 1) * BL],
                        rhs=wi[:],
                        start=(k == 0), stop=False)
                nc.tensor.matmul(
                    pst[0:BL, sl], lhsT=ones1[:],
                    rhs=binit[:, n * 512:(n + 1) * 512],
                    start=False, stop=True)
            h_cur = hc.tile([BL, H], F32, tag="h")
            nc.scalar.copy(h_cur[:], pst[0:BL, 0:512])
            c_cur = hc.tile([BL, H], F32, tag="c")
            nc.scalar.copy(c_cur[:], pst[0:BL, 512:1024])

            pst = ps.tile([128, GD], F32, tag="ps")
            hT = work.tile([128, 4 * BL], F32R, tag="hT")
            transpose_to(h_cur, hT, pst)
            hT16 = work.tile([128, 4 * BL], F16, tag="hT16")
            nc.vector.tensor_copy(hT16[:], hT[:])

            tok = small.tile([BL, 1], I32, tag="tok")
            nc.sync.dma_start(tok[:], tok0_d[:])
            ph0cm.__exit__(None, None, None)
            lgp = ctx.enter_context(tc.tile_pool(name="lgp", bufs=2))
            fld = ctx.enter_context(tc.tile_pool(name="fld", bufs=1))

            # ================= decode steps =================
            loop_cm = (tc.For_i(0, loop_reps) if loop_reps > 1
                       else contextlib.nullcontext())
            with loop_cm:
                for t in range(nsteps):
                    # --- xw = (emb@W_ih.T + b_ih + b_hh)[tok]  (row gather) ---
                    xw = work.tile([BL, GD], F32R, tag="xw")
                    nc.gpsimd.indirect_dma_start(
                        out=xw[:], out_offset=None, in_=xw_d[:],
                        in_offset=bass.IndirectOffsetOnAxis(ap=tok[:, :1], axis=0))
                    # actw: activation outputs early-step, wcand gather
                    # late-step (disjoint lifetimes share one buffer)
                    actw = work.tile([BL, 4 * WROWW], F32, tag="actw")

                    # --- gates = xw + h@W_hh.T in one PSUM (float32r) ---
                    pgt = ps.tile([128, GD], F32, tag="ps")
                    for n in range(4):
                        ns = slice(n * 512, (n + 1) * 512)
                        for k in range(4):
                            nc.tensor.matmul(
                                pgt[0:BL, ns],
                                lhsT=hT[:, k * BL:(k + 1) * BL],
                                rhs=whh[k][:, ns],
                                start=(k == 0), stop=False)
                        nc.tensor.matmul(
                            pgt[0:BL, ns], lhsT=ident_r[:],
                            rhs=xw[:, ns],
                            start=False, stop=True)

                    # --- pointwise LSTM (gate order i,f,g,o) ---
                    sig_if = actw[:, 0:1024]
                    nc.scalar.activation(sig_if, pgt[0:BL, 0:1024], AF.Sigmoid)
                    tng = actw[:, 1024:1536]
                    nc.scalar.activation(tng, pgt[0:BL, 1024:1536], AF.Tanh)
                    sgo = actw[:, 1536:2048]
                    nc.scalar.activation(sgo, pgt[0:BL, 1536:2048], AF.Sigmoid)

                    t1 = work.tile([BL, 512], F32, tag="t1")
                    nc.vector.tensor_mul(t1[:], sig_if[:, 0:512], tng)
                    t2 = work.tile([BL, 512], F32, tag="t2")
                    nc.vector.tensor_mul(t2[:], sig_if[:, 512:1024], c_cur[:])
                    c_new = hc.tile([BL, H], F32, tag="c")
                    nc.vector.tensor_add(c_new[:], t2[:], t1[:])
                    tc2 = work.tile([BL, 512], F32, tag="t1")
                    nc.scalar.activation(tc2[:], c_new[:], AF.Tanh)
                    h_new = hc.tile([BL, H], F32, tag="h")
                    nc.vector.tensor_mul(h_new[:], sgo, tc2[:])
                    c_cur = c_new

                    pst = ps.tile([128, GD], F32, tag="ps")
                    hT = work.tile([128, 4 * BL], F32R, tag="hT")
                    transpose_to(h_new, hT, pst)
                    hT16 = work.tile([128, 4 * BL], F16, tag="hT16")
                    nc.vector.tensor_copy(hT16[:], hT[:])

                    # --- FC scan vs resident fp8 table; fold to 32-col
                    #     block maxima, logits out to DRAM (+scratch) ---
                    ot = out_d[min(t, out_slots - 1)]
                    for g in range(NGROUP):
                        wlo = g * GROUPW
                        wwid = min(GROUPW, V - wlo)
                        nb = wwid // 32
                        pf = ps.tile([128, GD], F32, tag="ps")
                        co = 0
                        while co < wwid:
                            cw = min(CHUNK, wwid - co)
                            cs = slice(co, co + cw)
                            for k in range(4):
                                nc.tensor.matmul(
                                    pf[0:BL, cs],
                                    lhsT=hT16[:, k * BL:(k + 1) * BL],
                                    rhs=wfc8[k][:, wlo + co:wlo + co + cw],
                                    start=(k == 0), stop=(k == 3))
                            co += cw
                        Lseg = lgp.tile([BL, 64, 32], F16, tag="L")
                        nc.scalar.copy(Lseg[:, 0:nb, :], pf[0:BL, 0:wwid])
                        nc.sync.dma_start(
                            ot[:, wlo:wlo + wwid], Lseg[:, 0:nb, :])
                        nc.sync.dma_start(
                            scrv[:, wlo:wlo + wwid], Lseg[:, 0:nb, :])
                        # fold tree: 32 -> 1 per block (fp16, 2x TT mode)
                        # ping-pong: A holds levels 1 and 3, B holds 2 and 4
                        FA = fld.tile([BL, 64, 16], F16, tag="FA")
                        nc.vector.tensor_max(
                            FA[:, 0:nb, :], Lseg[:, 0:nb, 0:16],
                            Lseg[:, 0:nb, 16:32])
                        FB = fld.tile([BL, 64, 8], F16, tag="FB")
                        nc.vector.tensor_max(
                            FB[:, 0:nb, :], FA[:, 0:nb, 0:8], FA[:, 0:nb, 8:16])
                        nc.vector.tensor_max(
                            FA[:, 0:nb, 0:4], FB[:, 0:nb, 0:4], FB[:, 0:nb, 4:8])
                        nc.vector.tensor_max(
                            FB[:, 0:nb, 0:2], FA[:, 0:nb, 0:2], FA[:, 0:nb, 2:4])
                        nc.vector.tensor_max(
                            Mt[:, g * 64:g * 64 + nb, :], FB[:, 0:nb, 0:1],
                            FB[:, 0:nb, 1:2])
                        # fence: a write to Lseg waits for the scr DMA read,
                        # then propagates into the gather offsets
                        nc.gpsimd.memset(Lseg[:, 0:1, 0:1], 0.0)
                        nc.gpsimd.tensor_copy(fence[:, g:g + 1],
                                              Lseg[:, 0:1, 0:1])

                    # --- top-4 blocks via knockout on block maxima [BL,1024]
                    m8 = small.tile([BL, 8], F16, tag="m8")
                    nc.vector.max(out=m8[:], in_=Mt[:, :, :])
                    m4pad = small.tile([BL, 8], F16, tag="m4pad")
                    nc.vector.memset(m4pad[:], 49152.0)
                    nc.vector.tensor_copy(m4pad[:, 0:NCAND], m8[:, 0:NCAND])
                    cv2 = small.tile([BL, 1000], F16, tag="cv2")
                    nc.vector.match_replace(
                        out=cv2[:], in_to_replace=m4pad[:],
                        in_values=Mt[:, :, :], imm_value=-49152.0)
                    msk = small.tile([BL, 1000], F16, tag="msk")
                    nc.vector.tensor_scalar(
                        out=msk[:], in0=cv2[:], scalar1=-49152.0, scalar2=None,
                        op0=OP.is_equal)
                    sel = small.tile([BL, 1000], F16, tag="sel")
                    nc.vector.tensor_mul(sel[:], msk[:], payload[:])
                    w8 = small.tile([BL, 8], F16, tag="w8")
                    nc.vector.max(out=w8[:], in_=sel[:])
                    bid4 = small.tile([BL, NCAND], F32, tag="bid4")
                    nc.vector.tensor_scalar(
                        out=bid4[:], in0=w8[:, 0:NCAND], scalar1=-1.0,
                        scalar2=2048.0, op0=OP.mult, op1=OP.add)
                    # gather the 4 candidate 32-col blocks from DRAM scratch
                    fzs = small.tile([BL, 1], F32, tag="fzs")
                    nc.vector.tensor_reduce(
                        out=fzs[:], in_=fence[:], axis=mybir.AxisListType.X,
                        op=OP.add)
                    fz0 = small.tile([BL, 1], F32, tag="fz0")
                    nc.vector.tensor_scalar(
                        out=fz0[:], in0=fzs[:], scalar1=0.0, scalar2=None,
                        op0=OP.mult)
                    rowsf = small.tile([BL, NCAND], F32, tag="rowsf")
                    nc.vector.tensor_scalar(
                        out=rowsf[:], in0=bid4[:], scalar1=rowbase[:, 0:1],
                        scalar2=None, op0=OP.add)
                    nc.vector.tensor_scalar(
                        out=rowsf[:], in0=rowsf[:], scalar1=fz0[:, 0:1],
                        scalar2=None, op0=OP.add)
                    cidb = small.tile([BL, NCAND], I32, tag="cidb")
                    nc.vector.tensor_copy(cidb[:], rowsf[:])
                    blks = small.tile([BL, NCAND, 32], F16, tag="blks")
                    for j in range(NCAND):
                        nc.gpsimd.indirect_dma_start(
                            out=blks[:, j], out_offset=None, in_=scr_d[:],
                            in_offset=bass.IndirectOffsetOnAxis(
                                ap=cidb[:, j:j + 1], axis=0))
                    # in-block position of each candidate
                    posf = small.tile([BL, NCAND], F32, tag="posf")
                    for j in range(NCAND):
                        bm8 = small.tile([BL, 8], F16, tag="bm8")
                        nc.vector.max(out=bm8[:], in_=blks[:, j])
                        bfi = small.tile([BL, 8], U16, tag="bfi")
                        nc.vector.max_index(
                            out=bfi[:], in_max=bm8[:], in_values=blks[:, j])
                        nc.vector.tensor_copy(posf[:, j:j + 1], bfi[:, 0:1])
                    gid4 = small.tile([BL, NCAND], F32, tag="gid4")
                    nc.vector.tensor_scalar(
                        out=gid4[:], in0=bid4[:], scalar1=32.0, scalar2=None,
                        op0=OP.mult)
                    nc.vector.tensor_add(gid4[:], gid4[:], posf[:])
                    cid = small.tile([BL, NCAND], I32, tag="cid")
                    nc.vector.tensor_copy(cid[:], gid4[:])
                    wcand = actw
                    for j in range(NCAND):
                        nc.gpsimd.indirect_dma_start(
                            out=wcand[:, j * WROWW:(j + 1) * WROWW],
                            out_offset=None, in_=wrow_d[:],
                            in_offset=bass.IndirectOffsetOnAxis(
                                ap=cid[:, j:j + 1], axis=0))
                    # exact fp32 dots (+bias as reduce init) on DVE
                    ex = small.tile([BL, 8], F32, tag="ex")
                    nc.vector.memset(ex[:], -1e30)
                    prod = work.tile([BL, 512], F32, tag="prod")
                    for j in range(NCAND):
                        nc.vector.tensor_mul(
                            prod[:], wcand[:, j * WROWW:j * WROWW + 512],
                            h_new[:])
                        nc.vector.tensor_reduce(
                            out=ex[:, j:j + 1], in_=prod[:],
                            axis=mybir.AxisListType.X, op=OP.add)
                    for j in range(NCAND):
                        nc.vector.tensor_add(
                            ex[:, j:j + 1], ex[:, j:j + 1],
                            wcand[:, j * WROWW + 512:j * WROWW + 513])

                    # --- winner (value desc, then smallest id) -> next token ---
                    em8 = small.tile([BL, 8], F32, tag="em8")
                    nc.vector.max(out=em8[:], in_=ex[:])
                    emsk = small.tile([BL, NCAND], F32, tag="emsk")
                    nc.vector.tensor_scalar(
                        out=emsk[:], in0=ex[:, 0:NCAND], scalar1=em8[:, 0:1],
                        scalar2=None, op0=OP.is_equal)
                    egneg = small.tile([BL, NCAND], F32, tag="egneg")
                    nc.vector.tensor_scalar(
                        out=egneg[:], in0=gid4[:], scalar1=-1.0, scalar2=48000.0,
                        op0=OP.mult, op1=OP.add)
                    esel = small.tile([BL, 8], F32, tag="esel")
                    nc.vector.memset(esel[:], 0.0)
                    nc.vector.tensor_mul(esel[:, 0:NCAND], emsk[:], egneg[:])
                    ew8 = small.tile([BL, 8], F32, tag="ew8")
                    nc.vector.max(out=ew8[:], in_=esel[:])
                    tokf = small.tile([BL, 1], F32, tag="tokf")
                    nc.vector.tensor_scalar(
                        out=tokf[:], in0=ew8[:, 0:1], scalar1=-1.0, scalar2=48000.0,
                        op0=OP.mult, op1=OP.add)
                    tok = small.tile([BL, 1], I32, tag="tok")
                    nc.vector.tensor_copy(tok[:], tokf[:])

    nc.compile()
    return nc


def _prep_inputs(inputs):
    enc = np.ascontiguousarray(np.asarray(inputs["encoder_outputs"], np.float32))
    captions = np.asarray(inputs["captions"])
    emb = np.asarray(inputs["embedding"], np.float32)
    W_ih = np.asarray(inputs["W_ih"], np.float32)
    b_ih = np.asarray(inputs["b_ih"], np.float32)
    W_hh = np.asarray(inputs["W_hh"], np.float32)
    b_hh = np.asarray(inputs["b_hh"], np.float32)
    W_fc = np.asarray(inputs["W_fc"], np.float32)
    b_fc = np.asarray(inputs["b_fc"], np.float32)
    W_init_h = np.asarray(inputs["W_init_h"], np.float32)
    b_init_h = np.asarray(inputs["b_init_h"], np.float32)
    W_init_c = np.asarray(inputs["W_init_c"], np.float32)
    b_init_c = np.asarray(inputs["b_init_c"], np.float32)

    import ml_dtypes
    wfc8 = np.ascontiguousarray(
        np.clip(W_fc.T * np.float32(WSCALE), -15.5, 15.5)
        .reshape(4, 128, V)).astype(ml_dtypes.float8_e3m4)

    def trunc_r(a):
        return (a.view(np.uint32) & np.uint32(0xFFFFFC00)).view(np.float32)

    xw = np.ascontiguousarray(
        trunc_r((emb @ W_ih.T + b_ih + b_hh).astype(np.float32)))

    wrow = np.zeros((V, WROWW), np.float32)
    wrow[:, 0:512] = W_fc
    wrow[:, 512] = b_fc
    whhT = np.ascontiguousarray(trunc_r(W_hh.T).reshape(4, 128, GD))
    winitT = np.ascontiguousarray(
        (np.concatenate([W_init_h, W_init_c], axis=0) / np.float32(NPIX))
        .T.reshape(4, 128, 1024))
    binit = np.concatenate([b_init_h, b_init_c]).reshape(1, 1024)

    blkd = np.zeros((128, 13 * 8), np.float32)
    for k in range(13):
        for i in range(128):
            r = k * 128 + i
            if r < BL * NPIX:
                blkd[i, k * 8 + r // NPIX] = 1.0

    in_maps = []
    for c in range(NCORES):
        enc_c = enc[c * BL:(c + 1) * BL].reshape(BL * NPIX, H)
        enc_pad = np.zeros((13 * 128, H), np.float32)
        enc_pad[:BL * NPIX] = enc_c
        tok0 = np.ascontiguousarray(
            captions[c * BL:(c + 1) * BL, 0].astype(np.int32).reshape(BL, 1))
        in_maps.append({
            "wfc8": wfc8,
            "xw": xw,
            "wrow": wrow,
            "whhT": whhT,
            "winitT": winitT,
            "binit": binit,
            "enc": enc_pad.reshape(13, 128, H),
            "blkdiag": blkd,
            "tok0": tok0,
        })
    return in_maps


def kernel(**inputs) -> np.ndarray:
    if "nc" not in _CACHE:
        _CACHE["nc"] = _build_nc()
    nc = _CACHE["nc"]
    in_maps = _prep_inputs(inputs)
    res = run_bass_kernel_spmd(nc, in_maps, list(range(NCORES)))
    out = np.zeros((B, T, V), np.float32)
    inv = np.float32(1.0 / WSCALE)
    for c in range(NCORES):
        lg = res.results[c]["logits"][:NSTEPS]     # [31, BL, V] fp16 (x32)
        out[c * BL:(c + 1) * BL, 1:, :] = lg.transpose(1, 0, 2)
    out[:, 1:, :] *= inv
    b_fc = np.asarray(inputs["b_fc"], np.float32)
    if b_fc.any():
        # scan logits are computed without the FC bias (it is ordering-exact
        # in the on-chip recheck); fold it into the full output here
        out[:, 1:, :] += b_fc
    return out
